# revision 34
# baseline (speedup 1.0000x reference)
"""MaxK-GCN conv on 8 Trainium2 NeuronCores.

Pipeline (per core c, SPMD over 8 cores; nodes sharded 8 x 12500):
  phase 1: h = featT_c.T @ W (PE), top-16-of-64 threshold mask (DVE max8 +
           match_replace), scale by (max(out_deg,1)*max(in_deg,1))^-0.5, and
           split each fp32 row into a [hi|lo] bf16 pair -> local table shard
           [12544, 128] bf16 (hi+lo reconstructs fp32 to ~2^-17).
  AllGather table shards -> full table [100352, 128] bf16 in DRAM.
  phase 2: edges with dst in shard c, host-sorted by (pass, slab, block):
           dma_gather src rows (4 SWDGE queues), one-hot S tiles from dst
           values (DVE is_eq vs iota), matmul S^T @ G accumulating per
           128-dst-node block in PSUM (PE), fold hi+lo + bias (DVE), DMA out.

Edge bookkeeping (sort, padding, degree counts) is host-side index metadata;
all floating-point math runs on device.
"""
import sys
import os

sys.path.insert(0, "/opt/trn_rl_repo")

import numpy as np
import ml_dtypes
import concourse.bacc as bacc
import concourse.mybir as mybir
import concourse.tile as tile
from concourse.bass_utils import run_bass_kernel_spmd

P = 128
N_NODES = 100000
IN_FEATS = 256
OUT_FEATS = 64
N_CORES = 8
SHARD = N_NODES // N_CORES          # 12500 real nodes per core
SHARD_PAD = 12544                   # 98 * 128
N_BLOCKS = SHARD_PAD // P           # 98
N_SLABS = 4
QROWS = SHARD_PAD // 4              # 3136 rows per shard-quarter
SLAB_ROWS = N_CORES * QROWS         # 25088 rows per quarter-table
PASS_BLOCKS = [4] * 24 + [2]        # blocks per PSUM pass (1 block = 1 bank)
N_PASSES = len(PASS_BLOCKS)
TABLE_ROWS = N_CORES * SHARD_PAD    # 100352
PAIR = 2 * OUT_FEATS                # 128 bf16 per table row (hi|lo)
TILES_PER_CALL = 16                 # balanced (pass, slab) group size
NEG_INF = -3.0e38

PASS_OF_BLOCK = np.repeat(np.arange(N_PASSES), PASS_BLOCKS)
PASS_BASE = np.cumsum([0] + PASS_BLOCKS[:-1])


def _balance_perms(src, dst, in_deg, out_deg):
    """Load-balancing layout: per-core dst->block LPT + src->quarter greedy.

    Returns (dst_pos, src_pos): position of each node within its core's
    padded shard, for the output rows (dst_pos) and table rows (src_pos).
    Balancing makes every (core, slab, block) bucket fit exactly
    ceil(~2045/4/128) = 4 tiles, shrinking the shared tile grid ~18%.
    """
    import heapq

    dst_pos = np.zeros(N_NODES, dtype=np.int64)
    for c in range(N_CORES):
        lo = c * SHARD
        deg = in_deg[lo:lo + SHARD]
        order = np.argsort(-deg, kind="stable")
        loads = np.zeros(N_BLOCKS, dtype=np.int64)
        slots = np.zeros(N_BLOCKS, dtype=np.int64)
        heap = [(0, b) for b in range(N_BLOCKS)]
        heapq.heapify(heap)
        pos = np.empty(SHARD, dtype=np.int64)
        for v in order:
            while True:
                _, b = heapq.heappop(heap)
                if slots[b] < P:
                    break
            pos[v] = b * P + slots[b]
            slots[b] += 1
            loads[b] += deg[v]
            if slots[b] < P:
                heapq.heappush(heap, (loads[b], b))
        dst_pos[lo:lo + SHARD] = pos

    # src->quarter greedy: balance each consumer bucket (dst core, block)
    # across the 4 quarter-tables; cells <= 512 keep buckets at 4 tiles.
    edge_bucket = ((dst // SHARD) * N_BLOCKS + (dst_pos[dst] // P)).astype(np.int64)
    nbkt = N_CORES * N_BLOCKS
    cell = np.zeros((nbkt, N_SLABS), dtype=np.int32)
    cap = 512
    src_quarter = np.zeros(N_NODES, dtype=np.int8)
    order_e = np.argsort(src, kind="stable")
    sb = edge_bucket[order_e]
    s_sorted = src[order_e]
    starts = np.searchsorted(s_sorted, np.arange(N_NODES))
    ends = np.searchsorted(s_sorted, np.arange(N_NODES) + 1)
    for c in range(N_CORES):
        lo = c * SHARD
        node_order = np.argsort(-out_deg[lo:lo + SHARD], kind="stable") + lo
        qcap = np.full(N_SLABS, QROWS, dtype=np.int64)
        for v in node_order:
            bkts = sb[starts[v]:ends[v]]
            if len(bkts):
                loads = cell[bkts]
                penalty = (np.maximum(loads + 1 - cap, 0) * 1000 + loads).sum(axis=0)
            else:
                penalty = np.zeros(N_SLABS)
            penalty = penalty + (qcap == 0) * 1e12
            q = int(np.argmin(penalty))
            src_quarter[v] = q
            qcap[q] -= 1
            if len(bkts):
                np.add.at(cell, (bkts, q), 1)

    # repair: move one contributor out of each overflowing cell when possible,
    # respecting per-core slab row capacities
    fill = np.zeros((N_CORES, N_SLABS), dtype=np.int64)
    for c in range(N_CORES):
        fill[c] = np.bincount(src_quarter[c * SHARD:(c + 1) * SHARD],
                              minlength=N_SLABS)
    eq_slab = src_quarter[src]
    for _ in range(3):
        over = np.argwhere(cell > cap)
        if not len(over):
            break
        for bkt, q in over:
            if cell[bkt, q] <= cap:
                continue
            cand = np.unique(src[(edge_bucket == bkt) & (eq_slab == q)])
            moved = False
            for v in cand:
                vc = int(v) // SHARD
                bkts = sb[starts[v]:ends[v]]
                for q2 in range(N_SLABS):
                    if q2 == q or fill[vc, q2] >= QROWS:
                        continue
                    ub, mult = np.unique(bkts, return_counts=True)
                    if np.all(cell[ub, q2] + mult <= cap):
                        np.add.at(cell, (bkts, q), -1)
                        np.add.at(cell, (bkts, q2), 1)
                        src_quarter[v] = q2
                        fill[vc, q] -= 1
                        fill[vc, q2] += 1
                        eq_slab = src_quarter[src]
                        moved = True
                        break
                if moved:
                    break

    # quarter capacity accounting was per-core in the greedy; repair may
    # overfill a quarter by a few rows -- verify and fall back if so
    src_pos = np.zeros(N_NODES, dtype=np.int64)
    for c in range(N_CORES):
        lo = c * SHARD
        qs = src_quarter[lo:lo + SHARD]
        fill = np.zeros(N_SLABS, dtype=np.int64)
        pos = np.empty(SHARD, dtype=np.int64)
        ok = np.bincount(qs, minlength=N_SLABS).max() <= QROWS
        if not ok:
            qs = np.repeat(np.arange(N_SLABS), QROWS)[:SHARD].astype(np.int8)
        for i in range(SHARD):
            q = int(qs[i])
            pos[i] = q * QROWS + fill[q]
            fill[q] += 1
        src_pos[lo:lo + SHARD] = pos
    return dst_pos, src_pos


def _inspect(src, dst, dst_pos, src_pos):
    """Per-core edge keys + shared static tile grid (order-independent)."""
    core = dst // SHARD
    e_blk = dst_pos[dst] >> 7
    e_rel = dst_pos[dst] & (P - 1)
    e_s8 = src // SHARD
    e_slab = src_pos[src] // QROWS
    e_gidx = e_s8 * QROWS + (src_pos[src] - e_slab * QROWS)
    per_core = []
    counts = np.zeros((N_CORES, N_PASSES, N_SLABS, N_BLOCKS), dtype=np.int64)
    for c in range(N_CORES):
        m = core == c
        blk = e_blk[m]
        slab = e_slab[m]
        gidx = e_gidx[m]
        rel = e_rel[m]
        pss = PASS_OF_BLOCK[blk]
        key = (pss * N_SLABS + slab) * N_BLOCKS + blk
        cnt = np.bincount(key, minlength=N_PASSES * N_SLABS * N_BLOCKS)
        counts[c] = cnt.reshape(N_PASSES, N_SLABS, N_BLOCKS)
        per_core.append((blk, slab, gidx, rel, pss))
    T = ((counts + P - 1) // P).max(axis=0)  # shared tile grid
    return per_core, counts, T


def _sort_streams(per_core, rank):
    """Sort each core's edges into the scheduled stream order."""
    gidx_of, dstrel_of = [], []
    for blk, slab, gidx, rel, pss in per_core:
        order = np.lexsort((gidx, blk, rank[pss, slab]))
        gidx_of.append(gidx[order])
        dstrel_of.append(rel[order])
    return gidx_of, dstrel_of


def _make_schedule(T):
    """Windowed-diagonal (pass, slab) stream: prefetch early slabs of future
    passes while waiting for later AllGather chunks, with a pass window W
    bounded by the PSUM accumulator pool."""
    W = 4
    order = []
    ptr = [0] * N_SLABS
    completed = 0
    while len(order) < N_PASSES * N_SLABS:
        best = None
        for s in range(N_SLABS):
            if ptr[s] < N_PASSES and ptr[s] < completed + W:
                best = (ptr[s], s)
                break
        if best is None:
            for s in range(N_SLABS):
                if ptr[s] < N_PASSES:
                    best = (ptr[s], s)
                    break
        order.append(best)
        ptr[best[1]] += 1
        while completed < N_PASSES and all(pt > completed for pt in ptr):
            completed += 1

    tile_meta = []   # [p, s, b, start, stop]
    calls = []       # (s, j0, ct) one per (p, s) group
    pos = {}
    for (p, s) in order:
        j0 = len(tile_meta)
        for b in range(PASS_BASE[p], PASS_BASE[p] + PASS_BLOCKS[p]):
            for k in range(int(T[p, s, b])):
                tile_meta.append([p, s, b, False, False])
        calls.append((s, j0, len(tile_meta) - j0))
    # start/stop per (p, b) across the whole stream
    first, last = {}, {}
    for j, (p, s, b, _, _) in enumerate(tile_meta):
        first.setdefault(b, j)
        last[b] = j
    for b, j in first.items():
        tile_meta[j][3] = True
    for b, j in last.items():
        tile_meta[j][4] = True
    rank = np.zeros((N_PASSES, N_SLABS), dtype=np.int64)
    for i, (p, s) in enumerate(order):
        rank[p, s] = i
    return tile_meta, calls, order, rank


def _per_core_streams(c, tile_meta, counts, gidx_of, dstrel_of):
    """This core's padded gather-idx + dst_rel streams matching the grid."""
    ntiles = len(tile_meta)
    idx_stream = np.zeros(ntiles * P, dtype=np.int16)
    dst_stream = np.full(ntiles * P, -1.0, dtype=np.float32)
    edge_ptr = 0
    j = 0
    while j < ntiles:
        p, s, b = tile_meta[j][:3]
        k = j
        while k < ntiles and tile_meta[k][:3] == [p, s, b]:
            k += 1
        nseg = int(counts[c, p, s, b])
        base = j * P
        idx_stream[base:base + nseg] = gidx_of[c][edge_ptr:edge_ptr + nseg]
        dst_stream[base:base + nseg] = dstrel_of[c][edge_ptr:edge_ptr + nseg]
        edge_ptr += nseg
        j = k
    assert edge_ptr == len(gidx_of[c])
    idx_wrapped = np.tile(idx_stream.reshape(-1, 16).T, (8, 1)).copy()
    dstv = dst_stream.reshape(ntiles, P).T.copy()
    return idx_wrapped, dstv


def _build(tile_meta, calls):
    ntiles = len(tile_meta)
    nc = bacc.Bacc("TRN2", target_bir_lowering=False, num_swdge_queues=4)
    dt = mybir.dt

    featT = nc.declare_dram_parameter("featT", [IN_FEATS, SHARD_PAD], dt.float32, isOutput=False)
    w_in = nc.declare_dram_parameter("w", [IN_FEATS, OUT_FEATS], dt.float32, isOutput=False)
    biasb = nc.declare_dram_parameter("biasb", [P, OUT_FEATS], dt.float32, isOutput=False)
    idegw = nc.declare_dram_parameter("idegw", [P, N_BLOCKS], dt.float32, isOutput=False)
    odegw = nc.declare_dram_parameter("odegw", [P, N_BLOCKS], dt.float32, isOutput=False)
    iota_in = nc.declare_dram_parameter("iota", [P, P], dt.bfloat16, isOutput=False)
    idxs_in = nc.declare_dram_parameter("idxs", [P, ntiles * 8], dt.int16, isOutput=False)
    dstv_in = nc.declare_dram_parameter("dstv", [P, ntiles], dt.bfloat16, isOutput=False)
    out_d = nc.declare_dram_parameter("out", [SHARD_PAD, OUT_FEATS], dt.float32, isOutput=True)

    tableL = nc.dram_tensor("tableL", [SHARD_PAD, PAIR], dt.bfloat16)
    tableQ = [nc.dram_tensor(f"tableQ{q}", [SLAB_ROWS, PAIR], dt.bfloat16,
                             addr_space="Shared") for q in range(N_SLABS)]

    with tile.TileContext(nc) as tc:
        with tc.tile_pool(name="const", bufs=1) as constp, \
             tc.tile_pool(name="gp", bufs=20) as gp, \
             tc.tile_pool(name="sp", bufs=10) as sps, \
             tc.tile_pool(name="outp", bufs=4) as outp:

            # ---- constants ----
            w_sb = constp.tile([P, 2, OUT_FEATS], dt.float32)
            for k in range(2):
                nc.sync.dma_start(out=w_sb[:, k, :], in_=w_in[k * P:(k + 1) * P, :])
            bias_sb = constp.tile([P, OUT_FEATS], dt.float32)
            nc.sync.dma_start(out=bias_sb[:], in_=biasb[:])
            iota_sb = constp.tile([P, 1, P], dt.bfloat16)
            nc.sync.dma_start(out=iota_sb[:, 0, :], in_=iota_in[:])
            dstv_sb = constp.tile([P, ntiles, 1], dt.bfloat16)
            nc.sync.dma_start(out=dstv_sb[:, :, 0], in_=dstv_in[:])
            idx_sb = constp.tile([P, ntiles * 8], dt.int16)
            nc.sync.dma_start(out=idx_sb[:], in_=idxs_in[:])

            # ---- phase 1: table build (pools scoped to free SBUF/PSUM) ----
            with tc.tile_pool(name="ft", bufs=1) as ftp, \
                 tc.tile_pool(name="ph1", bufs=4) as ph1, \
                 tc.tile_pool(name="ph1ps", bufs=4, space="PSUM") as ph1ps:

                ideg_sb = ph1.tile([P, N_BLOCKS], dt.float32, tag="deg")
                odeg_sb = ph1.tile([P, N_BLOCKS], dt.float32, tag="deg")
                nc.sync.dma_start(out=ideg_sb[:], in_=idegw[:])
                nc.sync.dma_start(out=odeg_sb[:], in_=odegw[:])
                scale_sb = constp.tile([P, N_BLOCKS], dt.float32)
                nc.vector.tensor_scalar_max(ideg_sb[:], ideg_sb[:], 1.0)
                nc.vector.tensor_scalar_max(odeg_sb[:], odeg_sb[:], 1.0)
                nc.vector.tensor_mul(out=scale_sb[:], in0=ideg_sb[:], in1=odeg_sb[:])
                nc.scalar.activation(out=scale_sb[:], in_=scale_sb[:],
                                     func=mybir.ActivationFunctionType.Sqrt)
                nc.vector.reciprocal(out=scale_sb[:], in_=scale_sb[:])

                # featT in 25-tile chunks (2 k-chunks x 4 column chunks)
                FCH = [13] * 7 + [7]
                FBASE = [0, 13, 26, 39, 52, 65, 78, 91]
                ft_sb = {}
                for fc in range(8):
                    for k in range(2):
                        t_ = ftp.tile([P, FCH[fc] * P], dt.float32, tag=f"ft{k}", bufs=2)
                        nc.sync.dma_start(
                            out=t_[:],
                            in_=featT[k * P:(k + 1) * P,
                                      FBASE[fc] * P:(FBASE[fc] + FCH[fc]) * P])
                        ft_sb[(fc, k)] = t_

                for t in range(N_BLOCKS):
                    fc = min(t // 13, 7)
                    tc_rel = t - FBASE[fc]
                    hp = ph1ps.tile([P, OUT_FEATS], dt.float32, tag="hps")
                    for k in range(2):
                        nc.tensor.matmul(
                            out=hp[:],
                            lhsT=ft_sb[(fc, k)][:, tc_rel * P:(tc_rel + 1) * P],
                            rhs=w_sb[:, k, :],
                            start=(k == 0), stop=(k == 1),
                        )
                    h = ph1.tile([P, OUT_FEATS], dt.float32, tag="h")
                    nc.vector.tensor_copy(out=h[:], in_=hp[:])
                    m1 = ph1.tile([P, 8], dt.float32, tag="m1")
                    nc.vector.max(m1[:], h[:])
                    hneg = ph1.tile([P, OUT_FEATS], dt.float32, tag="hneg")
                    nc.vector.match_replace(out=hneg[:], in_to_replace=m1[:],
                                            in_values=h[:], imm_value=NEG_INF)
                    m2 = ph1.tile([P, 8], dt.float32, tag="m2")
                    nc.vector.max(m2[:], hneg[:])
                    # hm = (h >= thr) * h  in one fused op
                    hm = ph1.tile([P, OUT_FEATS], dt.float32, tag="mask")
                    nc.vector.scalar_tensor_tensor(
                        out=hm[:], in0=h[:], scalar=m2[:, 7:8], in1=h[:],
                        op0=mybir.AluOpType.is_ge, op1=mybir.AluOpType.mult)
                    ttile = ph1.tile([P, PAIR], dt.bfloat16, tag="ttile")
                    hi32 = ph1.tile([P, OUT_FEATS], dt.float32, tag="hi32")
                    # hi = bf16(hm * scale) via ACT's fused input scale
                    nc.scalar.activation(out=ttile[:, 0:OUT_FEATS], in_=hm[:],
                                         func=mybir.ActivationFunctionType.Copy,
                                         scale=scale_sb[:, t:t + 1])
                    nc.scalar.activation(out=hi32[:], in_=ttile[:, 0:OUT_FEATS],
                                         func=mybir.ActivationFunctionType.Copy)
                    # lo = bf16(hm * scale - hi32) in one fused op
                    nc.vector.scalar_tensor_tensor(
                        out=ttile[:, OUT_FEATS:PAIR], in0=hm[:],
                        scalar=scale_sb[:, t:t + 1], in1=hi32[:],
                        op0=mybir.AluOpType.mult,
                        op1=mybir.AluOpType.subtract)
                    nc.sync.dma_start(out=tableL[t * P:(t + 1) * P, :], in_=ttile[:])

            # ---- allgather table, one collective per shard-quarter so
            # phase-2 gathers can start before phase 1 fully drains ----
            for q in range(N_SLABS):
                nc.gpsimd.collective_compute(
                    "AllGather",
                    mybir.AluOpType.bypass,
                    replica_groups=[list(range(N_CORES))],
                    ins=[tableL[q * QROWS:(q + 1) * QROWS, :]],
                    outs=[tableQ[q][:]],
                )

            # ---- phase 2: edge aggregation ----
            phase2_stack = __import__("contextlib").ExitStack()
            accp = phase2_stack.enter_context(
                tc.tile_pool(name="accp", bufs=6, space="PSUM"))
            g_tiles = {}
            for ci, (s, j0, ct) in enumerate(calls):
                g = gp.tile([P, TILES_PER_CALL, PAIR], dt.bfloat16, tag="g")
                nc.gpsimd.dma_gather(
                    out_ap=g[:, :ct, :],
                    in_ap=tableQ[s][:],
                    idxs_ap=idx_sb[:, j0 * 8:(j0 + ct) * 8],
                    num_idxs=ct * P,
                    num_idxs_reg=ct * P,
                    elem_size=PAIR,
                    single_packet=False,
                    queue_num=s,
                )
                for t in range(ct):
                    g_tiles[j0 + t] = (g, t)

            SW = 8

            # Gathers and S-builds follow the windowed stream; each pass's
            # matmuls are emitted block-major at pass completion so that
            # accumulation groups sharing a PSUM bank are strictly
            # sequential (HW allows one open group per bank).
            remaining = {}
            pending = {}
            for p_, s_, b_, _, _ in tile_meta:
                remaining[p_] = remaining.get(p_, 0) + 1
            for j, (p, s, b, st, sp_) in enumerate(tile_meta):
                pending.setdefault(p, []).append((j, b))
                remaining[p] -= 1
                if remaining[p]:
                    continue
                acc = accp.tile([P, 4, PAIR], dt.float32, tag="acc",
                                name=f"acc{p}")
                ostage = outp.tile([P, 4, OUT_FEATS], dt.float32,
                                   tag="ostage", name=f"ostage{p}")
                # lazy batched one-hot builds for this pass's tiles
                s_tiles = {}
                js = sorted(j2 for (j2, _) in pending[p])
                for i0 in range(0, len(js), SW):
                    grp = js[i0:i0 + SW]
                    # tiles of a (p, s) group are contiguous in the stream
                    j0g = grp[0]
                    jn = len(grp)
                    assert grp == list(range(j0g, j0g + jn))
                    s4 = sps.tile([P, SW, P], dt.bfloat16, tag="s",
                                  name=f"s{p}_{i0}")
                    nc.vector.tensor_tensor(
                        out=s4[:, :jn, :],
                        in0=dstv_sb[:, j0g:j0g + jn, :].to_broadcast([P, jn, P]),
                        in1=iota_sb[:].to_broadcast([P, jn, P]),
                        op=mybir.AluOpType.is_equal)
                    for t in range(jn):
                        s_tiles[j0g + t] = (s4, t)
                for b2 in range(PASS_BASE[p], PASS_BASE[p] + PASS_BLOCKS[p]):
                    lst = [j2 for (j2, bb) in pending[p] if bb == b2]
                    b_rel = b2 - PASS_BASE[p]
                    for i, j2 in enumerate(lst):
                        s4_t, s4_i = s_tiles[j2]
                        g, gt = g_tiles[j2]
                        nc.tensor.matmul(
                            out=acc[:, b_rel, :],
                            lhsT=s4_t[:, s4_i, :],
                            rhs=g[:, gt, :],
                            start=(i == 0), stop=(i == len(lst) - 1),
                            skip_group_check=True,
                        )
                    nc.vector.tensor_add(out=ostage[:, b_rel, :],
                                         in0=acc[:, b_rel, 0:OUT_FEATS],
                                         in1=bias_sb[:])
                    nc.vector.tensor_add(out=ostage[:, b_rel, :],
                                         in0=ostage[:, b_rel, :],
                                         in1=acc[:, b_rel, OUT_FEATS:PAIR])
                    nc.sync.dma_start(out=out_d[b2 * P:(b2 + 1) * P, :],
                                      in_=ostage[:, b_rel, :])
            phase2_stack.close()

    nc.finalize()
    return nc


def kernel(feat, weight, bias, src, dst):
    feat = np.asarray(feat, dtype=np.float32)
    weight = np.asarray(weight, dtype=np.float32)
    bias = np.asarray(bias, dtype=np.float32)
    src = np.asarray(src)
    dst = np.asarray(dst)

    src64 = src.astype(np.int64)
    dst64 = dst.astype(np.int64)
    in_deg = np.bincount(dst64, minlength=N_NODES).astype(np.float32)
    out_deg = np.bincount(src64, minlength=N_NODES).astype(np.float32)
    dst_pos, src_pos = _balance_perms(src64, dst64,
                                      np.bincount(dst64, minlength=N_NODES),
                                      np.bincount(src64, minlength=N_NODES))
    per_core, counts, T = _inspect(src64, dst64, dst_pos, src_pos)
    tile_meta, calls, order, rank = _make_schedule(T)
    gidx_of, dstrel_of = _sort_streams(per_core, rank)

    ft = feat.T  # [256, 100000]
    iota = np.tile(np.arange(P, dtype=np.float32), (P, 1)).astype(ml_dtypes.bfloat16)

    in_maps = []
    for c in range(N_CORES):
        lo, hi = c * SHARD, (c + 1) * SHARD
        featT_c = np.zeros((IN_FEATS, SHARD_PAD), dtype=np.float32)
        featT_c[:, src_pos[lo:hi]] = ft[:, lo:hi]
        ideg_c = np.ones(SHARD_PAD, dtype=np.float32)
        odeg_c = np.ones(SHARD_PAD, dtype=np.float32)
        ideg_c[src_pos[lo:hi]] = in_deg[lo:hi]
        odeg_c[src_pos[lo:hi]] = out_deg[lo:hi]
        idx_wrapped, dstv = _per_core_streams(c, tile_meta, counts,
                                              gidx_of, dstrel_of)
        in_maps.append({
            "featT": featT_c,
            "w": weight,
            "biasb": np.tile(bias[None, :], (P, 1)).astype(np.float32),
            "idegw": ideg_c.reshape(N_BLOCKS, P).T.copy(),
            "odegw": odeg_c.reshape(N_BLOCKS, P).T.copy(),
            "iota": iota,
            "idxs": idx_wrapped,
            "dstv": dstv.astype(ml_dtypes.bfloat16),
        })

    nc = _build(tile_meta, calls)
    res = run_bass_kernel_spmd(nc, in_maps, list(range(N_CORES)),
                               trace=bool(os.environ.get("KERNEL_TRACE")))
    if os.environ.get("KERNEL_TRACE"):
        print(f"HW exec time: {res.exec_time_ns} ns")
    out = np.empty((N_NODES, OUT_FEATS), dtype=np.float32)
    for c in range(N_CORES):
        lo, hi = c * SHARD, (c + 1) * SHARD
        out[lo:hi] = res.results[c]["out"][dst_pos[lo:hi]]
    return out



# revision 35
# speedup vs baseline: 1.0018x; 1.0018x over previous
"""MaxK-GCN conv on 8 Trainium2 NeuronCores.

Pipeline (per core c, SPMD over 8 cores; nodes sharded 8 x 12500):
  phase 1: h = featT_c.T @ W (PE), top-16-of-64 threshold mask (DVE max8 +
           match_replace), scale by (max(out_deg,1)*max(in_deg,1))^-0.5, and
           split each fp32 row into a [hi|lo] bf16 pair -> local table shard
           [12544, 128] bf16 (hi+lo reconstructs fp32 to ~2^-17).
  AllGather table shards -> full table [100352, 128] bf16 in DRAM.
  phase 2: edges with dst in shard c, host-sorted by (pass, slab, block):
           dma_gather src rows (4 SWDGE queues), one-hot S tiles from dst
           values (DVE is_eq vs iota), matmul S^T @ G accumulating per
           128-dst-node block in PSUM (PE), fold hi+lo + bias (DVE), DMA out.

Edge bookkeeping (sort, padding, degree counts) is host-side index metadata;
all floating-point math runs on device.
"""
import sys
import os

sys.path.insert(0, "/opt/trn_rl_repo")

import numpy as np
import ml_dtypes
import concourse.bacc as bacc
import concourse.mybir as mybir
import concourse.tile as tile
from concourse.bass_utils import run_bass_kernel_spmd

P = 128
N_NODES = 100000
IN_FEATS = 256
OUT_FEATS = 64
N_CORES = 8
SHARD = N_NODES // N_CORES          # 12500 real nodes per core
SHARD_PAD = 12544                   # 98 * 128
N_BLOCKS = SHARD_PAD // P           # 98
N_SLABS = 4
QROWS = SHARD_PAD // 4              # 3136 rows per shard-quarter
SLAB_ROWS = N_CORES * QROWS         # 25088 rows per quarter-table
PASS_BLOCKS = [4] * 24 + [2]        # blocks per PSUM pass (1 block = 1 bank)
N_PASSES = len(PASS_BLOCKS)
TABLE_ROWS = N_CORES * SHARD_PAD    # 100352
PAIR = 2 * OUT_FEATS                # 128 bf16 per table row (hi|lo)
TILES_PER_CALL = 16                 # balanced (pass, slab) group size
NEG_INF = -3.0e38

PASS_OF_BLOCK = np.repeat(np.arange(N_PASSES), PASS_BLOCKS)
PASS_BASE = np.cumsum([0] + PASS_BLOCKS[:-1])


def _balance_perms(src, dst, in_deg, out_deg):
    """Load-balancing layout: per-core dst->block LPT + src->quarter greedy.

    Returns (dst_pos, src_pos): position of each node within its core's
    padded shard, for the output rows (dst_pos) and table rows (src_pos).
    Balancing makes every (core, slab, block) bucket fit exactly
    ceil(~2045/4/128) = 4 tiles, shrinking the shared tile grid ~18%.
    """
    import heapq

    dst_pos = np.zeros(N_NODES, dtype=np.int64)
    for c in range(N_CORES):
        lo = c * SHARD
        deg = in_deg[lo:lo + SHARD]
        order = np.argsort(-deg, kind="stable")
        loads = np.zeros(N_BLOCKS, dtype=np.int64)
        slots = np.zeros(N_BLOCKS, dtype=np.int64)
        heap = [(0, b) for b in range(N_BLOCKS)]
        heapq.heapify(heap)
        pos = np.empty(SHARD, dtype=np.int64)
        for v in order:
            while True:
                _, b = heapq.heappop(heap)
                if slots[b] < P:
                    break
            pos[v] = b * P + slots[b]
            slots[b] += 1
            loads[b] += deg[v]
            if slots[b] < P:
                heapq.heappush(heap, (loads[b], b))
        dst_pos[lo:lo + SHARD] = pos

    # src->quarter greedy: balance each consumer bucket (dst core, block)
    # across the 4 quarter-tables; cells <= 512 keep buckets at 4 tiles.
    edge_bucket = ((dst // SHARD) * N_BLOCKS + (dst_pos[dst] // P)).astype(np.int64)
    nbkt = N_CORES * N_BLOCKS
    cell = np.zeros((nbkt, N_SLABS), dtype=np.int32)
    cap = 512
    src_quarter = np.zeros(N_NODES, dtype=np.int8)
    order_e = np.argsort(src, kind="stable")
    sb = edge_bucket[order_e]
    s_sorted = src[order_e]
    starts = np.searchsorted(s_sorted, np.arange(N_NODES))
    ends = np.searchsorted(s_sorted, np.arange(N_NODES) + 1)
    for c in range(N_CORES):
        lo = c * SHARD
        node_order = np.argsort(-out_deg[lo:lo + SHARD], kind="stable") + lo
        qcap = np.full(N_SLABS, QROWS, dtype=np.int64)
        for v in node_order:
            bkts = sb[starts[v]:ends[v]]
            if len(bkts):
                loads = cell[bkts]
                penalty = (np.maximum(loads + 1 - cap, 0) * 1000 + loads).sum(axis=0)
            else:
                penalty = np.zeros(N_SLABS)
            penalty = penalty + (qcap == 0) * 1e12
            q = int(np.argmin(penalty))
            src_quarter[v] = q
            qcap[q] -= 1
            if len(bkts):
                np.add.at(cell, (bkts, q), 1)

    # repair: move one contributor out of each overflowing cell when possible,
    # respecting per-core slab row capacities
    fill = np.zeros((N_CORES, N_SLABS), dtype=np.int64)
    for c in range(N_CORES):
        fill[c] = np.bincount(src_quarter[c * SHARD:(c + 1) * SHARD],
                              minlength=N_SLABS)
    eq_slab = src_quarter[src]
    for _ in range(3):
        over = np.argwhere(cell > cap)
        if not len(over):
            break
        for bkt, q in over:
            if cell[bkt, q] <= cap:
                continue
            cand = np.unique(src[(edge_bucket == bkt) & (eq_slab == q)])
            moved = False
            for v in cand:
                vc = int(v) // SHARD
                bkts = sb[starts[v]:ends[v]]
                for q2 in range(N_SLABS):
                    if q2 == q or fill[vc, q2] >= QROWS:
                        continue
                    ub, mult = np.unique(bkts, return_counts=True)
                    if np.all(cell[ub, q2] + mult <= cap):
                        np.add.at(cell, (bkts, q), -1)
                        np.add.at(cell, (bkts, q2), 1)
                        src_quarter[v] = q2
                        fill[vc, q] -= 1
                        fill[vc, q2] += 1
                        eq_slab = src_quarter[src]
                        moved = True
                        break
                if moved:
                    break

    # quarter capacity accounting was per-core in the greedy; repair may
    # overfill a quarter by a few rows -- verify and fall back if so
    src_pos = np.zeros(N_NODES, dtype=np.int64)
    for c in range(N_CORES):
        lo = c * SHARD
        qs = src_quarter[lo:lo + SHARD]
        fill = np.zeros(N_SLABS, dtype=np.int64)
        pos = np.empty(SHARD, dtype=np.int64)
        ok = np.bincount(qs, minlength=N_SLABS).max() <= QROWS
        if not ok:
            qs = np.repeat(np.arange(N_SLABS), QROWS)[:SHARD].astype(np.int8)
        for i in range(SHARD):
            q = int(qs[i])
            pos[i] = q * QROWS + fill[q]
            fill[q] += 1
        src_pos[lo:lo + SHARD] = pos
    return dst_pos, src_pos


def _inspect(src, dst, dst_pos, src_pos):
    """Per-core edge keys + shared static tile grid (order-independent)."""
    core = dst // SHARD
    e_blk = dst_pos[dst] >> 7
    e_rel = dst_pos[dst] & (P - 1)
    e_s8 = src // SHARD
    e_slab = src_pos[src] // QROWS
    e_gidx = e_s8 * QROWS + (src_pos[src] - e_slab * QROWS)
    per_core = []
    counts = np.zeros((N_CORES, N_PASSES, N_SLABS, N_BLOCKS), dtype=np.int64)
    for c in range(N_CORES):
        m = core == c
        blk = e_blk[m]
        slab = e_slab[m]
        gidx = e_gidx[m]
        rel = e_rel[m]
        pss = PASS_OF_BLOCK[blk]
        key = (pss * N_SLABS + slab) * N_BLOCKS + blk
        cnt = np.bincount(key, minlength=N_PASSES * N_SLABS * N_BLOCKS)
        counts[c] = cnt.reshape(N_PASSES, N_SLABS, N_BLOCKS)
        per_core.append((blk, slab, gidx, rel, pss))
    T = ((counts + P - 1) // P).max(axis=0)  # shared tile grid
    return per_core, counts, T


def _sort_streams(per_core, rank):
    """Sort each core's edges into the scheduled stream order."""
    gidx_of, dstrel_of = [], []
    for blk, slab, gidx, rel, pss in per_core:
        order = np.lexsort((gidx, blk, rank[pss, slab]))
        gidx_of.append(gidx[order])
        dstrel_of.append(rel[order])
    return gidx_of, dstrel_of


def _make_schedule(T):
    """Windowed-diagonal (pass, slab) stream: prefetch early slabs of future
    passes while waiting for later AllGather chunks, with a pass window W
    bounded by the PSUM accumulator pool."""
    W = 4
    order = []
    ptr = [0] * N_SLABS
    completed = 0
    while len(order) < N_PASSES * N_SLABS:
        best = None
        for s in range(N_SLABS):
            if ptr[s] < N_PASSES and ptr[s] < completed + W:
                best = (ptr[s], s)
                break
        if best is None:
            for s in range(N_SLABS):
                if ptr[s] < N_PASSES:
                    best = (ptr[s], s)
                    break
        order.append(best)
        ptr[best[1]] += 1
        while completed < N_PASSES and all(pt > completed for pt in ptr):
            completed += 1

    tile_meta = []   # [p, s, b, start, stop]
    calls = []       # (s, j0, ct) one per (p, s) group
    pos = {}
    for (p, s) in order:
        j0 = len(tile_meta)
        for b in range(PASS_BASE[p], PASS_BASE[p] + PASS_BLOCKS[p]):
            for k in range(int(T[p, s, b])):
                tile_meta.append([p, s, b, False, False])
        calls.append((s, j0, len(tile_meta) - j0))
    # start/stop per (p, b) across the whole stream
    first, last = {}, {}
    for j, (p, s, b, _, _) in enumerate(tile_meta):
        first.setdefault(b, j)
        last[b] = j
    for b, j in first.items():
        tile_meta[j][3] = True
    for b, j in last.items():
        tile_meta[j][4] = True
    rank = np.zeros((N_PASSES, N_SLABS), dtype=np.int64)
    for i, (p, s) in enumerate(order):
        rank[p, s] = i
    return tile_meta, calls, order, rank


def _per_core_streams(c, tile_meta, counts, gidx_of, dstrel_of):
    """This core's padded gather-idx + dst_rel streams matching the grid."""
    ntiles = len(tile_meta)
    idx_stream = np.zeros(ntiles * P, dtype=np.int16)
    dst_stream = np.full(ntiles * P, -1.0, dtype=np.float32)
    edge_ptr = 0
    j = 0
    while j < ntiles:
        p, s, b = tile_meta[j][:3]
        k = j
        while k < ntiles and tile_meta[k][:3] == [p, s, b]:
            k += 1
        nseg = int(counts[c, p, s, b])
        base = j * P
        idx_stream[base:base + nseg] = gidx_of[c][edge_ptr:edge_ptr + nseg]
        dst_stream[base:base + nseg] = dstrel_of[c][edge_ptr:edge_ptr + nseg]
        edge_ptr += nseg
        j = k
    assert edge_ptr == len(gidx_of[c])
    idx_wrapped = np.tile(idx_stream.reshape(-1, 16).T, (8, 1)).copy()
    dstv = dst_stream.reshape(ntiles, P).T.copy()
    return idx_wrapped, dstv


def _build(tile_meta, calls):
    ntiles = len(tile_meta)
    nc = bacc.Bacc("TRN2", target_bir_lowering=False, num_swdge_queues=4)
    dt = mybir.dt

    featT = nc.declare_dram_parameter("featT", [IN_FEATS, SHARD_PAD], dt.float32, isOutput=False)
    w_in = nc.declare_dram_parameter("w", [IN_FEATS, OUT_FEATS], dt.float32, isOutput=False)
    biasb = nc.declare_dram_parameter("biasb", [P, OUT_FEATS], dt.float32, isOutput=False)
    idegw = nc.declare_dram_parameter("idegw", [P, N_BLOCKS], dt.float32, isOutput=False)
    odegw = nc.declare_dram_parameter("odegw", [P, N_BLOCKS], dt.float32, isOutput=False)
    iota_in = nc.declare_dram_parameter("iota", [P, P], dt.bfloat16, isOutput=False)
    idxs_in = nc.declare_dram_parameter("idxs", [P, ntiles * 8], dt.int16, isOutput=False)
    dstv_in = nc.declare_dram_parameter("dstv", [P, ntiles], dt.bfloat16, isOutput=False)
    out_d = nc.declare_dram_parameter("out", [SHARD_PAD, OUT_FEATS], dt.float32, isOutput=True)

    tableL = nc.dram_tensor("tableL", [SHARD_PAD, PAIR], dt.bfloat16)
    tableQ = [nc.dram_tensor(f"tableQ{q}", [SLAB_ROWS, PAIR], dt.bfloat16,
                             addr_space="Shared") for q in range(N_SLABS)]

    with tile.TileContext(nc) as tc:
        with tc.tile_pool(name="const", bufs=1) as constp, \
             tc.tile_pool(name="gp", bufs=20) as gp, \
             tc.tile_pool(name="sp", bufs=10) as sps, \
             tc.tile_pool(name="outp", bufs=4) as outp:

            # ---- constants ----
            w_sb = constp.tile([P, 2, OUT_FEATS], dt.float32)
            for k in range(2):
                nc.sync.dma_start(out=w_sb[:, k, :], in_=w_in[k * P:(k + 1) * P, :])
            bias_sb = constp.tile([P, OUT_FEATS], dt.float32)
            nc.sync.dma_start(out=bias_sb[:], in_=biasb[:])
            iota_sb = constp.tile([P, 1, P], dt.bfloat16)
            nc.sync.dma_start(out=iota_sb[:, 0, :], in_=iota_in[:])
            dstv_sb = constp.tile([P, ntiles, 1], dt.bfloat16)
            idx_sb = constp.tile([P, ntiles * 8], dt.int16)

            # ---- phase 1: table build (pools scoped to free SBUF/PSUM) ----
            with tc.tile_pool(name="ft", bufs=1) as ftp, \
                 tc.tile_pool(name="ph1", bufs=4) as ph1, \
                 tc.tile_pool(name="ph1ps", bufs=4, space="PSUM") as ph1ps:

                ideg_sb = ph1.tile([P, N_BLOCKS], dt.float32, tag="deg")
                odeg_sb = ph1.tile([P, N_BLOCKS], dt.float32, tag="deg")
                nc.sync.dma_start(out=ideg_sb[:], in_=idegw[:])
                nc.sync.dma_start(out=odeg_sb[:], in_=odegw[:])
                scale_sb = constp.tile([P, N_BLOCKS], dt.float32)
                nc.vector.tensor_scalar_max(ideg_sb[:], ideg_sb[:], 1.0)
                nc.vector.tensor_scalar_max(odeg_sb[:], odeg_sb[:], 1.0)
                nc.vector.tensor_mul(out=scale_sb[:], in0=ideg_sb[:], in1=odeg_sb[:])
                nc.scalar.activation(out=scale_sb[:], in_=scale_sb[:],
                                     func=mybir.ActivationFunctionType.Sqrt)
                nc.vector.reciprocal(out=scale_sb[:], in_=scale_sb[:])

                # featT in 25-tile chunks (2 k-chunks x 4 column chunks)
                FCH = [13] * 7 + [7]
                FBASE = [0, 13, 26, 39, 52, 65, 78, 91]
                ft_sb = {}
                for fc in range(8):
                    for k in range(2):
                        t_ = ftp.tile([P, FCH[fc] * P], dt.float32, tag=f"ft{k}", bufs=2)
                        nc.sync.dma_start(
                            out=t_[:],
                            in_=featT[k * P:(k + 1) * P,
                                      FBASE[fc] * P:(FBASE[fc] + FCH[fc]) * P])
                        ft_sb[(fc, k)] = t_

                # phase-2 index streams: loaded after the featT chunks so
                # they don't delay the AllGather-0 critical path; still far
                # ahead of the first gather (~124us)
                nc.sync.dma_start(out=dstv_sb[:, :, 0], in_=dstv_in[:])
                nc.sync.dma_start(out=idx_sb[:], in_=idxs_in[:])

                for t in range(N_BLOCKS):
                    fc = min(t // 13, 7)
                    tc_rel = t - FBASE[fc]
                    hp = ph1ps.tile([P, OUT_FEATS], dt.float32, tag="hps")
                    for k in range(2):
                        nc.tensor.matmul(
                            out=hp[:],
                            lhsT=ft_sb[(fc, k)][:, tc_rel * P:(tc_rel + 1) * P],
                            rhs=w_sb[:, k, :],
                            start=(k == 0), stop=(k == 1),
                        )
                    h = ph1.tile([P, OUT_FEATS], dt.float32, tag="h")
                    nc.vector.tensor_copy(out=h[:], in_=hp[:])
                    m1 = ph1.tile([P, 8], dt.float32, tag="m1")
                    nc.vector.max(m1[:], h[:])
                    hneg = ph1.tile([P, OUT_FEATS], dt.float32, tag="hneg")
                    nc.vector.match_replace(out=hneg[:], in_to_replace=m1[:],
                                            in_values=h[:], imm_value=NEG_INF)
                    m2 = ph1.tile([P, 8], dt.float32, tag="m2")
                    nc.vector.max(m2[:], hneg[:])
                    # hm = (h >= thr) * h  in one fused op
                    hm = ph1.tile([P, OUT_FEATS], dt.float32, tag="mask")
                    nc.vector.scalar_tensor_tensor(
                        out=hm[:], in0=h[:], scalar=m2[:, 7:8], in1=h[:],
                        op0=mybir.AluOpType.is_ge, op1=mybir.AluOpType.mult)
                    ttile = ph1.tile([P, PAIR], dt.bfloat16, tag="ttile")
                    hi32 = ph1.tile([P, OUT_FEATS], dt.float32, tag="hi32")
                    # hi = bf16(hm * scale) via ACT's fused input scale
                    nc.scalar.activation(out=ttile[:, 0:OUT_FEATS], in_=hm[:],
                                         func=mybir.ActivationFunctionType.Copy,
                                         scale=scale_sb[:, t:t + 1])
                    nc.scalar.activation(out=hi32[:], in_=ttile[:, 0:OUT_FEATS],
                                         func=mybir.ActivationFunctionType.Copy)
                    # lo = bf16(hm * scale - hi32) in one fused op
                    nc.vector.scalar_tensor_tensor(
                        out=ttile[:, OUT_FEATS:PAIR], in0=hm[:],
                        scalar=scale_sb[:, t:t + 1], in1=hi32[:],
                        op0=mybir.AluOpType.mult,
                        op1=mybir.AluOpType.subtract)
                    nc.sync.dma_start(out=tableL[t * P:(t + 1) * P, :], in_=ttile[:])

            # ---- allgather table, one collective per shard-quarter so
            # phase-2 gathers can start before phase 1 fully drains ----
            for q in range(N_SLABS):
                nc.gpsimd.collective_compute(
                    "AllGather",
                    mybir.AluOpType.bypass,
                    replica_groups=[list(range(N_CORES))],
                    ins=[tableL[q * QROWS:(q + 1) * QROWS, :]],
                    outs=[tableQ[q][:]],
                )

            # ---- phase 2: edge aggregation ----
            phase2_stack = __import__("contextlib").ExitStack()
            accp = phase2_stack.enter_context(
                tc.tile_pool(name="accp", bufs=6, space="PSUM"))
            g_tiles = {}
            for ci, (s, j0, ct) in enumerate(calls):
                g = gp.tile([P, TILES_PER_CALL, PAIR], dt.bfloat16, tag="g")
                nc.gpsimd.dma_gather(
                    out_ap=g[:, :ct, :],
                    in_ap=tableQ[s][:],
                    idxs_ap=idx_sb[:, j0 * 8:(j0 + ct) * 8],
                    num_idxs=ct * P,
                    num_idxs_reg=ct * P,
                    elem_size=PAIR,
                    single_packet=False,
                    queue_num=s,
                )
                for t in range(ct):
                    g_tiles[j0 + t] = (g, t)

            SW = 8

            # Gathers and S-builds follow the windowed stream; each pass's
            # matmuls are emitted block-major at pass completion so that
            # accumulation groups sharing a PSUM bank are strictly
            # sequential (HW allows one open group per bank).
            remaining = {}
            pending = {}
            for p_, s_, b_, _, _ in tile_meta:
                remaining[p_] = remaining.get(p_, 0) + 1
            for j, (p, s, b, st, sp_) in enumerate(tile_meta):
                pending.setdefault(p, []).append((j, b))
                remaining[p] -= 1
                if remaining[p]:
                    continue
                acc = accp.tile([P, 4, PAIR], dt.float32, tag="acc",
                                name=f"acc{p}")
                ostage = outp.tile([P, 4, OUT_FEATS], dt.float32,
                                   tag="ostage", name=f"ostage{p}")
                # lazy batched one-hot builds for this pass's tiles
                s_tiles = {}
                js = sorted(j2 for (j2, _) in pending[p])
                for i0 in range(0, len(js), SW):
                    grp = js[i0:i0 + SW]
                    # tiles of a (p, s) group are contiguous in the stream
                    j0g = grp[0]
                    jn = len(grp)
                    assert grp == list(range(j0g, j0g + jn))
                    s4 = sps.tile([P, SW, P], dt.bfloat16, tag="s",
                                  name=f"s{p}_{i0}")
                    nc.vector.tensor_tensor(
                        out=s4[:, :jn, :],
                        in0=dstv_sb[:, j0g:j0g + jn, :].to_broadcast([P, jn, P]),
                        in1=iota_sb[:].to_broadcast([P, jn, P]),
                        op=mybir.AluOpType.is_equal)
                    for t in range(jn):
                        s_tiles[j0g + t] = (s4, t)
                for b2 in range(PASS_BASE[p], PASS_BASE[p] + PASS_BLOCKS[p]):
                    lst = [j2 for (j2, bb) in pending[p] if bb == b2]
                    b_rel = b2 - PASS_BASE[p]
                    for i, j2 in enumerate(lst):
                        s4_t, s4_i = s_tiles[j2]
                        g, gt = g_tiles[j2]
                        nc.tensor.matmul(
                            out=acc[:, b_rel, :],
                            lhsT=s4_t[:, s4_i, :],
                            rhs=g[:, gt, :],
                            start=(i == 0), stop=(i == len(lst) - 1),
                            skip_group_check=True,
                        )
                    nc.vector.tensor_add(out=ostage[:, b_rel, :],
                                         in0=acc[:, b_rel, 0:OUT_FEATS],
                                         in1=bias_sb[:])
                    nc.vector.tensor_add(out=ostage[:, b_rel, :],
                                         in0=ostage[:, b_rel, :],
                                         in1=acc[:, b_rel, OUT_FEATS:PAIR])
                    nc.sync.dma_start(out=out_d[b2 * P:(b2 + 1) * P, :],
                                      in_=ostage[:, b_rel, :])
            phase2_stack.close()

    nc.finalize()
    return nc


def kernel(feat, weight, bias, src, dst):
    feat = np.asarray(feat, dtype=np.float32)
    weight = np.asarray(weight, dtype=np.float32)
    bias = np.asarray(bias, dtype=np.float32)
    src = np.asarray(src)
    dst = np.asarray(dst)

    src64 = src.astype(np.int64)
    dst64 = dst.astype(np.int64)
    in_deg = np.bincount(dst64, minlength=N_NODES).astype(np.float32)
    out_deg = np.bincount(src64, minlength=N_NODES).astype(np.float32)
    dst_pos, src_pos = _balance_perms(src64, dst64,
                                      np.bincount(dst64, minlength=N_NODES),
                                      np.bincount(src64, minlength=N_NODES))
    per_core, counts, T = _inspect(src64, dst64, dst_pos, src_pos)
    tile_meta, calls, order, rank = _make_schedule(T)
    gidx_of, dstrel_of = _sort_streams(per_core, rank)

    ft = feat.T  # [256, 100000]
    iota = np.tile(np.arange(P, dtype=np.float32), (P, 1)).astype(ml_dtypes.bfloat16)

    in_maps = []
    for c in range(N_CORES):
        lo, hi = c * SHARD, (c + 1) * SHARD
        featT_c = np.zeros((IN_FEATS, SHARD_PAD), dtype=np.float32)
        featT_c[:, src_pos[lo:hi]] = ft[:, lo:hi]
        ideg_c = np.ones(SHARD_PAD, dtype=np.float32)
        odeg_c = np.ones(SHARD_PAD, dtype=np.float32)
        ideg_c[src_pos[lo:hi]] = in_deg[lo:hi]
        odeg_c[src_pos[lo:hi]] = out_deg[lo:hi]
        idx_wrapped, dstv = _per_core_streams(c, tile_meta, counts,
                                              gidx_of, dstrel_of)
        in_maps.append({
            "featT": featT_c,
            "w": weight,
            "biasb": np.tile(bias[None, :], (P, 1)).astype(np.float32),
            "idegw": ideg_c.reshape(N_BLOCKS, P).T.copy(),
            "odegw": odeg_c.reshape(N_BLOCKS, P).T.copy(),
            "iota": iota,
            "idxs": idx_wrapped,
            "dstv": dstv.astype(ml_dtypes.bfloat16),
        })

    nc = _build(tile_meta, calls)
    res = run_bass_kernel_spmd(nc, in_maps, list(range(N_CORES)),
                               trace=bool(os.environ.get("KERNEL_TRACE")))
    if os.environ.get("KERNEL_TRACE"):
        print(f"HW exec time: {res.exec_time_ns} ns")
    out = np.empty((N_NODES, OUT_FEATS), dtype=np.float32)
    for c in range(N_CORES):
        lo, hi = c * SHARD, (c + 1) * SHARD
        out[lo:hi] = res.results[c]["out"][dst_pos[lo:hi]]
    return out

